# revision 11
# baseline (speedup 1.0000x reference)
"""Multi-head attention (B=8, N=1024, C=768, H=12, D=64) on 8 trn2 NeuronCores.

Sharding: pure data-parallel — one batch element per core, weights replicated,
no collectives.

Per-core dataflow (all matmuls in float32r = full-rate ~FP22 precision):
  phase 1: qkT[1536,1024] = w_qkv[:, :1536].T @ x        (x fed pre-transposed)
  phase 2: v[1024,768]    = x @ w_qkv[:, 1536:], stored per-head with an
           appended ones column (v' = [v_h | 1]) so the attn@v matmul also
           produces the softmax denominator as output row 64.
  phase 3: per head-pair b, query-tile qt:
           S^T chunks [128k, 512q] via two K=64 matmuls (row-packed heads),
           exp on ScalarE, U' = v'.T @ E accumulated over k-chunks,
           normalize by row-64 reciprocal broadcast via a tiny K=2 matmul.
  phase 4: y = O @ w_proj  (+ b_proj added on host; it is zeros anyway).

Scale 1/sqrt(64) is folded into the q columns of w_qkv on the host.
"""

import numpy as np

_CACHE = {}

B, N, C = 8, 1024, 768
H, D = 12, 64
NCORES = 8

last_exec_time_ns = None


MT_G = N // 128


def _build():
    import concourse.bacc as bacc
    import concourse.tile as tile
    from concourse import mybir

    F32 = mybir.dt.float32
    F32R = mybir.dt.float32r
    EXP = mybir.ActivationFunctionType.Exp

    def r(ap):
        return ap if ap.dtype == F32R else ap.bitcast(F32R)

    nc = bacc.Bacc(
        "TRN2", target_bir_lowering=False, debug=False, num_devices=NCORES
    )

    xt_d = nc.dram_tensor("xt", [C, N], F32R, kind="ExternalInput")
    wqkv_d = nc.dram_tensor("wqkv", [C, 3 * C], F32R, kind="ExternalInput")
    wproj_d = nc.dram_tensor("wproj", [C, C], F32R, kind="ExternalInput")
    sel_d = nc.dram_tensor("sel", [2, 128], F32R, kind="ExternalInput")
    ones_d = nc.dram_tensor("ones", [128, MT_G, H, 1], F32R, kind="ExternalInput")
    y_d = nc.dram_tensor("y", [N, C], F32, kind="ExternalOutput")

    FC = C // 128  # 6 feature chunks
    MT = N // 128  # 8 token tiles
    QT = N // 512  # 2 query tiles
    KC = N // 128  # 8 key chunks
    HB = H // 2  # 6 head-pair blocks

    with tile.TileContext(nc) as tc:
        with (
            tc.tile_pool(name="const", bufs=1) as const_p,
            tc.tile_pool(name="qk", bufs=1) as qk_p,
            tc.tile_pool(name="vv", bufs=1) as vv_p,
        ):
            sel = const_p.tile([2, 128], F32R, tag="sel")
            nc.sync.dma_start(sel[:], sel_d[:])

            # qkT[p, j, n]: row-block j of [q^T; k^T] (1536 x 1024)
            # q^T in blocks 0..5 (head h at block h//2, rows (h%2)*64..),
            # k^T in blocks 6..11.
            qkT = qk_p.tile([128, 12, N], F32R, tag="qkT")
            # vP[p, t, h, e]: v' per token-chunk t, head h, e in [0,65)
            vP = vv_p.tile([128, MT, H, D + 1], F32R, tag="vP")
            nc.sync.dma_start(vP[:, :, :, D : D + 1], ones_d[:])

            # ---------------- phases 1 + 2 ----------------
            with (
                tc.tile_pool(name="xw", bufs=1) as xw_p,
                tc.tile_pool(name="ps12", bufs=4, space="PSUM") as ps12,
            ):
                xt = xw_p.tile([128, FC, N], F32R, tag="xt")
                wq = xw_p.tile([128, FC, 3 * C], F32R, tag="wq")
                for fc in range(FC):
                    nc.sync.dma_start(xt[:, fc, :], xt_d[fc * 128 : (fc + 1) * 128, :])
                    nc.sync.dma_start(
                        wq[:, fc, :], wqkv_d[fc * 128 : (fc + 1) * 128, :]
                    )

                for m in range(12):
                    for qt in range(QT):
                        ps = ps12.tile([128, 512], F32, tag="ps_qk")
                        for fc in range(FC):
                            nc.tensor.matmul(
                                ps[:],
                                r(wq[:, fc, m * 128 : (m + 1) * 128]),
                                r(xt[:, fc, qt * 512 : (qt + 1) * 512]),
                                start=(fc == 0),
                                stop=(fc == FC - 1),
                            )
                        nc.vector.tensor_copy(
                            qkT[:, m, qt * 512 : (qt + 1) * 512], ps[:]
                        )

                for t in range(MT):
                    for nt in range(2):
                        ps = ps12.tile([128, 384], F32, tag="ps_v")
                        for fc in range(FC):
                            nc.tensor.matmul(
                                ps[:],
                                r(xt[:, fc, t * 128 : (t + 1) * 128]),
                                r(wq[:, fc, 2 * C + nt * 384 : 2 * C + (nt + 1) * 384]),
                                start=(fc == 0),
                                stop=(fc == FC - 1),
                            )
                        nc.vector.tensor_copy(
                            vP[:, t, nt * 6 : (nt + 1) * 6, 0:D],
                            ps[:].rearrange("p (h e) -> p h e", e=D),
                        )

            # oT[p, fcb, n]: attention output transposed (768 x 1024)
            with (
                tc.tile_pool(name="ot", bufs=1) as ot_p,
                tc.tile_pool(name="wp", bufs=1) as wp_p,
            ):
                oT = ot_p.tile([128, FC, N], F32R, tag="oT")
                wp = wp_p.tile([128, FC, C], F32R, tag="wp")
                for fc in range(FC):
                    nc.sync.dma_start(
                        wp[:, fc, :], wproj_d[fc * 128 : (fc + 1) * 128, :]
                    )

                # ---------------- phase 3 ----------------
                with (
                    tc.tile_pool(name="eP", bufs=2) as e_p,
                    tc.tile_pool(name="st3", bufs=3) as st3_p,
                    tc.tile_pool(name="s_ps", bufs=2, space="PSUM") as s_ps,
                    tc.tile_pool(name="u_ps", bufs=1, space="PSUM") as u_ps,
                    tc.tile_pool(name="r_ps", bufs=2, space="PSUM") as r_ps,
                ):
                    for b in range(HB):
                        for qt in range(QT):
                            qsl = slice(qt * 512, (qt + 1) * 512)
                            EA = e_p.tile([128, KC, 512], F32R, tag="EA")
                            EB = e_p.tile([128, KC, 512], F32R, tag="EB")
                            for kc in range(KC):
                                sA = s_ps.tile([128, 512], F32, tag="sA")
                                nc.tensor.matmul(
                                    sA[:],
                                    r(qkT[0:64, HB + b, kc * 128 : (kc + 1) * 128]),
                                    r(qkT[0:64, b, qsl]),
                                    start=True,
                                    stop=True,
                                )
                                sB = s_ps.tile([128, 512], F32, tag="sB")
                                nc.tensor.matmul(
                                    sB[:],
                                    r(qkT[64:128, HB + b, kc * 128 : (kc + 1) * 128]),
                                    r(qkT[64:128, b, qsl]),
                                    start=True,
                                    stop=True,
                                )
                                nc.scalar.activation(EA[:, kc, :], sA[:], EXP)
                                nc.scalar.activation(EB[:, kc, :], sB[:], EXP)
                            uA = u_ps.tile([D + 1, 512], F32, tag="uA")
                            uB = u_ps.tile([D + 1, 512], F32, tag="uB")
                            for kc in range(KC):
                                nc.tensor.matmul(
                                    uA[:],
                                    r(vP[:, kc, 2 * b, :]),
                                    r(EA[:, kc, :]),
                                    start=(kc == 0),
                                    stop=(kc == KC - 1),
                                )
                                nc.tensor.matmul(
                                    uB[:],
                                    r(vP[:, kc, 2 * b + 1, :]),
                                    r(EB[:, kc, :]),
                                    start=(kc == 0),
                                    stop=(kc == KC - 1),
                                )
                            rd2 = st3_p.tile([2, 512], F32R, tag="rd2")
                            rstA = st3_p.tile([D + 1, 512], F32R, tag="rstA")
                            with nc.allow_low_precision(reason="fp32r softmax denom"):
                                nc.vector.reciprocal(
                                    rstA[D : D + 1, :], uA[D : D + 1, :]
                                )
                            nc.sync.dma_start(rd2[0:1, :], rstA[D : D + 1, :])
                            rstB = st3_p.tile([D + 1, 512], F32R, tag="rstB")
                            with nc.allow_low_precision(reason="fp32r softmax denom"):
                                nc.vector.reciprocal(
                                    rstB[D : D + 1, :], uB[D : D + 1, :]
                                )
                            nc.sync.dma_start(rd2[1:2, :], rstB[D : D + 1, :])

                            nc.vector.tensor_copy(oT[0:D, b, qsl], uA[0:D, :])
                            ustg = st3_p.tile([D, 512], F32R, tag="ustg")
                            nc.vector.tensor_copy(ustg[:], uB[0:D, :])
                            nc.sync.dma_start(oT[D:128, b, qsl], ustg[:])

                            rps = r_ps.tile([128, 512], F32, tag="rps")
                            nc.tensor.matmul(
                                rps[:], r(sel[:]), r(rd2[:]), start=True, stop=True
                            )
                            nc.vector.tensor_mul(
                                oT[:, b, qsl], oT[:, b, qsl], rps[:]
                            )

                # ---------------- phase 4 ----------------
                with (
                    tc.tile_pool(name="yo", bufs=3) as y_p,
                    tc.tile_pool(name="y_ps", bufs=4, space="PSUM") as y_ps,
                ):
                    for mt in range(MT):
                        yt = y_p.tile([128, C], F32, tag="yt")
                        for n0, nsz in ((0, 512), (512, 256)):
                            ps = y_ps.tile([128, nsz], F32, tag="y_ps")
                            for fc in range(FC):
                                nc.tensor.matmul(
                                    ps[:],
                                    r(oT[:, fc, mt * 128 : (mt + 1) * 128]),
                                    r(wp[:, fc, n0 : n0 + nsz]),
                                    start=(fc == 0),
                                    stop=(fc == FC - 1),
                                )
                            nc.vector.tensor_copy(yt[:, n0 : n0 + nsz], ps[:])
                        nc.sync.dma_start(y_d[mt * 128 : (mt + 1) * 128, :], yt[:])

    nc.compile()
    return nc


def _get_nc():
    if "nc" not in _CACHE:
        _CACHE["nc"] = _build()
    return _CACHE["nc"]


def _sel_const():
    sel = np.zeros((2, 128), dtype=np.float32)
    sel[0, 0:D] = 1.0
    sel[1, D:128] = 1.0
    return sel


def make_in_maps(x, w_qkv, w_proj):
    scale = np.float32(D ** -0.5)
    wqkv_s = np.ascontiguousarray(w_qkv, dtype=np.float32).copy()
    wqkv_s[:, :C] *= scale
    wproj = np.ascontiguousarray(w_proj, dtype=np.float32)
    sel = _sel_const()
    ones = np.ones((128, N // 128, H, 1), dtype=np.float32)
    return [
        {
            "xt": np.ascontiguousarray(x[i].T),
            "wqkv": wqkv_s,
            "wproj": wproj,
            "sel": sel,
            "ones": ones,
        }
        for i in range(NCORES)
    ]


def kernel(x, w_qkv, w_proj, b_proj):
    global last_exec_time_ns
    from concourse.bass_utils import run_bass_kernel_spmd

    nc = _get_nc()
    in_maps = make_in_maps(x, w_qkv, w_proj)
    res = run_bass_kernel_spmd(nc, in_maps, list(range(NCORES)))
    last_exec_time_ns = res.exec_time_ns

    out = np.stack([res.results[i]["y"] for i in range(NCORES)], axis=0)
    out = out + np.asarray(b_proj, dtype=np.float32)[None, None, :]
    return out


# revision 13
# speedup vs baseline: 1.5371x; 1.5371x over previous
"""Multi-head attention (B=8, N=1024, C=768, H=12, D=64) on 8 trn2 NeuronCores.

Sharding: pure data-parallel — one batch element per core, weights replicated,
no collectives.

Per-core dataflow (all matmuls in float32r = full-rate ~FP22 precision):
  phase 1: qkT[1536,1024] = w_qkv[:, :1536].T @ x        (x fed pre-transposed)
  phase 2: v[1024,768]    = x @ w_qkv[:, 1536:], stored per-head with an
           appended ones column (v' = [v_h | 1]) so the attn@v matmul also
           produces the softmax denominator as output row 64.
  phase 3: per head-pair b, query-tile qt:
           S^T chunks [128k, 512q] via two K=64 matmuls (row-packed heads),
           exp on ScalarE, U' = v'.T @ E accumulated over k-chunks,
           normalize by row-64 reciprocal broadcast via a tiny K=2 matmul.
  phase 4: y = O @ w_proj  (+ b_proj added on host; it is zeros anyway).

Scale 1/sqrt(64) is folded into the q columns of w_qkv on the host.
"""

import numpy as np

_CACHE = {}

B, N, C = 8, 1024, 768
H, D = 12, 64
NCORES = 8

last_exec_time_ns = None


def _emit_body(nc, tc, tile, mybir, dram, rep):
    F32 = mybir.dt.float32
    F32R = mybir.dt.float32r
    F16 = mybir.dt.float16
    EXP = mybir.ActivationFunctionType.Exp
    xt_d, wqkv_d, wproj_d, sel_d, ones_d, y_d = dram

    def r(ap):
        # fp16 operands pass through; true-f32 APs are bitcast to f32r
        return ap.bitcast(F32R) if ap.dtype == F32 else ap

    FC = C // 128  # 6 feature chunks
    MT = N // 128  # 8 token tiles
    QT = N // 512  # 2 query tiles
    KC = N // 128  # 8 key chunks
    HB = H // 2  # 6 head-pair blocks

    with (
        tc.tile_pool(name=f"const{rep}", bufs=1) as const_p,
        tc.tile_pool(name=f"qk{rep}", bufs=1) as qk_p,
        tc.tile_pool(name=f"vv{rep}", bufs=1) as vv_p,
    ):
        sel = const_p.tile([2, 128], F32R, tag="sel")
        nc.sync.dma_start(sel[:], sel_d[:])

        # qkT[p, j, n]: row-block j of [q^T; k^T] (1536 x 1024)
        # q^T in blocks 0..5 (head h at block h//2, rows (h%2)*64..),
        # k^T in blocks 6..11.
        qkT = qk_p.tile([128, 12, N], F16, tag="qkT")
        # vP[p, t, h, e]: v' per token-chunk t, head h, e in [0,65)
        vP = vv_p.tile([128, MT, H, D + 1], F16, tag="vP")
        nc.sync.dma_start(vP[:, :, :, D : D + 1], ones_d[:])

        # ---------------- phases 1 + 2 ----------------
        with (
            tc.tile_pool(name=f"xw{rep}", bufs=1) as xw_p,
            tc.tile_pool(name=f"ps12{rep}", bufs=4, space="PSUM") as ps12,
        ):
            xt = xw_p.tile([128, FC, N], F16, tag="xt")
            wq = xw_p.tile([128, FC, 3 * C], F16, tag="wq")
            for fc in range(FC):
                nc.sync.dma_start(xt[:, fc, :], xt_d[fc * 128 : (fc + 1) * 128, :])
                nc.sync.dma_start(wq[:, fc, :], wqkv_d[fc * 128 : (fc + 1) * 128, :])

            for m in range(12):
                for qt in range(QT):
                    ps = ps12.tile([128, 512], F32, tag="ps_qk")
                    for fc in range(FC):
                        nc.tensor.matmul(
                            ps[:],
                            r(wq[:, fc, m * 128 : (m + 1) * 128]),
                            r(xt[:, fc, qt * 512 : (qt + 1) * 512]),
                            start=(fc == 0),
                            stop=(fc == FC - 1),
                        )
                    nc.vector.tensor_copy(qkT[:, m, qt * 512 : (qt + 1) * 512], ps[:])

            for t in range(MT):
                for nt in range(2):
                    ps = ps12.tile([128, 384], F32, tag="ps_v")
                    for fc in range(FC):
                        nc.tensor.matmul(
                            ps[:],
                            r(xt[:, fc, t * 128 : (t + 1) * 128]),
                            r(wq[:, fc, 2 * C + nt * 384 : 2 * C + (nt + 1) * 384]),
                            start=(fc == 0),
                            stop=(fc == FC - 1),
                        )
                    nc.vector.tensor_copy(
                        vP[:, t, nt * 6 : (nt + 1) * 6, 0:D],
                        ps[:].rearrange("p (h e) -> p h e", e=D),
                    )

        # oT[p, fcb, n]: attention output transposed (768 x 1024)
        with (
            tc.tile_pool(name=f"ot{rep}", bufs=1) as ot_p,
            tc.tile_pool(name=f"wp{rep}", bufs=1) as wp_p,
        ):
            oT = ot_p.tile([128, FC, N], F16, tag="oT")
            wp = wp_p.tile([128, FC, C], F16, tag="wp")
            for fc in range(FC):
                nc.sync.dma_start(wp[:, fc, :], wproj_d[fc * 128 : (fc + 1) * 128, :])

            # ---------------- phase 3 ----------------
            with (
                tc.tile_pool(name=f"eP{rep}", bufs=2) as e_p,
                tc.tile_pool(name=f"st3{rep}", bufs=3) as st3_p,
                tc.tile_pool(name=f"s_ps{rep}", bufs=2, space="PSUM") as s_ps,
                tc.tile_pool(name=f"u_ps{rep}", bufs=1, space="PSUM") as u_ps,
                tc.tile_pool(name=f"r_ps{rep}", bufs=2, space="PSUM") as r_ps,
            ):
                for b in range(HB):
                    for qt in range(QT):
                        qsl = slice(qt * 512, (qt + 1) * 512)
                        EA = e_p.tile([128, KC, 512], F16, tag="EA")
                        EB = e_p.tile([128, KC, 512], F16, tag="EB")
                        for kc in range(KC):
                            sA = s_ps.tile([128, 512], F32, tag="sA")
                            nc.tensor.matmul(
                                sA[:],
                                r(qkT[0:64, HB + b, kc * 128 : (kc + 1) * 128]),
                                r(qkT[0:64, b, qsl]),
                                start=True,
                                stop=True,
                            )
                            sB = s_ps.tile([128, 512], F32, tag="sB")
                            nc.tensor.matmul(
                                sB[:],
                                r(qkT[64:128, HB + b, kc * 128 : (kc + 1) * 128]),
                                r(qkT[64:128, b, qsl]),
                                start=True,
                                stop=True,
                            )
                            nc.scalar.activation(EA[:, kc, :], sA[:], EXP)
                            nc.scalar.activation(EB[:, kc, :], sB[:], EXP)
                        uA = u_ps.tile([D + 1, 512], F32, tag="uA")
                        uB = u_ps.tile([D + 1, 512], F32, tag="uB")
                        for kc in range(KC):
                            nc.tensor.matmul(
                                uA[:],
                                r(vP[:, kc, 2 * b, :]),
                                r(EA[:, kc, :]),
                                start=(kc == 0),
                                stop=(kc == KC - 1),
                            )
                            nc.tensor.matmul(
                                uB[:],
                                r(vP[:, kc, 2 * b + 1, :]),
                                r(EB[:, kc, :]),
                                start=(kc == 0),
                                stop=(kc == KC - 1),
                            )
                        rd2 = st3_p.tile([2, 512], F32R, tag="rd2")
                        rstA = st3_p.tile([D + 1, 512], F32R, tag="rstA")
                        rstB = st3_p.tile([D + 1, 512], F32R, tag="rstB")
                        with nc.allow_low_precision(reason="fp32r softmax denom"):
                            nc.vector.reciprocal(rstA[D : D + 1, :], uA[D : D + 1, :])
                            nc.vector.reciprocal(rstB[D : D + 1, :], uB[D : D + 1, :])
                        nc.sync.dma_start(rd2[0:1, :], rstA[D : D + 1, :])
                        nc.sync.dma_start(rd2[1:2, :], rstB[D : D + 1, :])

                        nc.vector.tensor_copy(oT[0:D, b, qsl], uA[0:D, :])
                        ustg = st3_p.tile([D, 512], F16, tag="ustg")
                        nc.vector.tensor_copy(ustg[:], uB[0:D, :])
                        nc.sync.dma_start(oT[D:128, b, qsl], ustg[:])

                        rps = r_ps.tile([128, 512], F32, tag="rps")
                        nc.tensor.matmul(
                            rps[:], r(sel[:]), r(rd2[:]), start=True, stop=True
                        )
                        with nc.allow_low_precision(reason="fp32r normalize"):
                            nc.vector.tensor_mul(oT[:, b, qsl], oT[:, b, qsl], rps[:])

            # ---------------- phase 4 ----------------
            with (
                tc.tile_pool(name=f"yo{rep}", bufs=3) as y_p,
                tc.tile_pool(name=f"y_ps{rep}", bufs=4, space="PSUM") as y_ps,
            ):
                for mt in range(MT):
                    yt = y_p.tile([128, C], F32, tag="yt")
                    for n0, nsz in ((0, 512), (512, 256)):
                        ps = y_ps.tile([128, nsz], F32, tag="y_ps")
                        for fc in range(FC):
                            nc.tensor.matmul(
                                ps[:],
                                r(oT[:, fc, mt * 128 : (mt + 1) * 128]),
                                r(wp[:, fc, n0 : n0 + nsz]),
                                start=(fc == 0),
                                stop=(fc == FC - 1),
                            )
                        nc.vector.tensor_copy(yt[:, n0 : n0 + nsz], ps[:])
                    nc.sync.dma_start(y_d[mt * 128 : (mt + 1) * 128, :], yt[:])


def _build(reps=1):
    import concourse.bacc as bacc
    import concourse.tile as tile
    from concourse import mybir

    F32 = mybir.dt.float32
    F32R = mybir.dt.float32r
    F16 = mybir.dt.float16

    nc = bacc.Bacc("TRN2", target_bir_lowering=False, debug=False, num_devices=NCORES)

    xt_d = nc.dram_tensor("xt", [C, N], F16, kind="ExternalInput")
    wqkv_d = nc.dram_tensor("wqkv", [C, 3 * C], F16, kind="ExternalInput")
    wproj_d = nc.dram_tensor("wproj", [C, C], F16, kind="ExternalInput")
    sel_d = nc.dram_tensor("sel", [2, 128], F32R, kind="ExternalInput")
    ones_d = nc.dram_tensor("ones", [128, N // 128, H, 1], F16, kind="ExternalInput")
    y_d = nc.dram_tensor("y", [N, C], F32, kind="ExternalOutput")
    dram = (xt_d, wqkv_d, wproj_d, sel_d, ones_d, y_d)

    with tile.TileContext(nc) as tc:
        for rep in range(reps):
            _emit_body(nc, tc, tile, mybir, dram, rep)

    nc.compile()
    return nc


def _get_nc(reps=1):
    key = ("nc", reps)
    if key not in _CACHE:
        _CACHE[key] = _build(reps)
    return _CACHE[key]


def _sel_const():
    sel = np.zeros((2, 128), dtype=np.float32)
    sel[0, 0:D] = 1.0
    sel[1, D:128] = 1.0
    return sel


def make_in_maps(x, w_qkv, w_proj):
    scale = np.float32(D ** -0.5)
    wqkv_s = np.ascontiguousarray(w_qkv, dtype=np.float32).copy()
    wqkv_s[:, :C] *= scale
    wqkv16 = wqkv_s.astype(np.float16)
    wproj16 = np.ascontiguousarray(w_proj, dtype=np.float32).astype(np.float16)
    sel = _sel_const()
    ones = np.ones((128, N // 128, H, 1), dtype=np.float16)
    return [
        {
            "xt": np.ascontiguousarray(x[i].T).astype(np.float16),
            "wqkv": wqkv16,
            "wproj": wproj16,
            "sel": sel,
            "ones": ones,
        }
        for i in range(NCORES)
    ]


def kernel(x, w_qkv, w_proj, b_proj):
    global last_exec_time_ns
    from concourse.bass_utils import run_bass_kernel_spmd

    nc = _get_nc()
    in_maps = make_in_maps(x, w_qkv, w_proj)
    res = run_bass_kernel_spmd(nc, in_maps, list(range(NCORES)))
    last_exec_time_ns = res.exec_time_ns

    out = np.stack([res.results[i]["y"] for i in range(NCORES)], axis=0)
    out = out + np.asarray(b_proj, dtype=np.float32)[None, None, :]
    return out
